# revision 11
# baseline (speedup 1.0000x reference)
"""Trainium2 Bass kernel for nn_EnsembleModel (grouped ensemble dot-product).

Computes out[b, g] = sum_n x[b, g, n] * W[g, n] + b[g] for
x: [16384, 368, 16] f32, W: [368, 16] f32, b: [368] f32.

Strategy: data-parallel over 8 NeuronCores (batch 16384 -> 8 x 2048).
Per core: batch rows on SBUF partitions (contiguous DMA). A custom DVE op
(MAC_SCAN: out = cumsum(x * w) along the free dim, one pass at 1 elem/cyc)
replaces the 2-pass mul+reduce; per-group sums are recovered as strided
differences of the cumulative sum, then bias.

The kernel is jointly DVE-bound (~100us of scans at 0.96 GHz) and
HBM-bound (~115us of x streaming at ~420 GB/s), so everything else is
kept off both paths:
 - W reaches the chip as two bf16 rows (hi+lo split, 23.6 KB) and is
   broadcast to all 128 partitions ON-CHIP: TensorE ones-matmuls
   accumulate hi+lo into PSUM (exact to ~2^-18), ACT copies to SBUF.
   (Replicating f32 W via HBM costs 3 MB = ~9us of stream time.)
 - Group-sum extraction runs on the idle GPSIMD engine, not the DVE:
   x tiles carry a 16-elem prefix whose *weights* are zero, so the
   cumulative sum starts at 0 and one 368-elem SUB yields all groups
   (prefix data itself may be garbage; only NaN/Inf must be avoided,
   hence a one-time memset per physical buffer).
 - Bias adds also run on GPSIMD; bias is replicated via HBM (188 KB).
 - Tiles 0/15 are processed in quarters fully on the DVE: tile 0 so
   compute starts as soon as the first W chunks land, tile 15 so the
   post-last-byte tail is one quarter-scan, not a full tile.
"""

import sys

for _p in ("/opt/trn_rl_repo", "/root/.axon_site/_ro/trn_rl_repo"):
    if _p not in sys.path:
        sys.path.append(_p)

import numpy as np
import ml_dtypes

import concourse.bacc as bacc
import concourse.bass as bass
import concourse.mybir as mybir
import concourse.tile as tile
from concourse.bass_utils import run_bass_kernel_spmd

BATCH = 16384
NGROUPS = 368
NMODELS = 16
NCORES = 8
BS = BATCH // NCORES          # 2048 batch rows per core
P = 128                       # SBUF partitions
F = NGROUPS * NMODELS         # 5888 x-elems per row
FP = F + NMODELS              # 5904 = 16-elem zero-weight prefix + x row
NTILES = BS // P              # 16 tiles of 128 rows

NQ = 4                        # quarters for the ramp tiles (0 and 15)
FQ = F // NQ                  # 1472
GQ = NGROUPS // NQ            # 92
MMCH = 512                    # matmul broadcast chunk (one PSUM bank of f32)
NXBUF = 5                     # persistent x buffers for the 14 full tiles

_CACHE = {}


def _register_mac_scan():
    """Register the fused multiply+cumsum custom DVE op at runtime."""
    import concourse.dve_ops as dve_ops
    from concourse.dve_ops import DveOp, OPS
    from concourse.dve_spec import AluOp, Spec, Src0, Src1, lower, scan
    from concourse.dve_spec import _has_src1 as has_src1
    from concourse.dve_uop import DveOpSpec

    name = "MAC_SCAN_ANT"
    for op in OPS:
        if op.name == name:
            return op

    def _ref(in0, in1, s0, s1, imm2):
        p = in0.shape[0]
        prod = (np.asarray(in0, np.float32) * np.asarray(in1, np.float32)).reshape(
            p, -1
        )
        return np.cumsum(prod, axis=1, dtype=np.float32).reshape(in0.shape)

    sha = {}
    op = DveOp(
        name,
        Spec(body=scan(AluOp.ADD, Src0 * Src1), reference=_ref),
        subdim=False,
        uops_sha=sha,
    )
    OPS.append(op)
    opcode = dve_ops._CUSTOM_DVE_ROW_BASE + len(OPS) - 1
    dve_ops._SUB_OPCODE_FOR_NAME[name] = opcode
    assert opcode < 0x20
    for ver in ("v3", "v4"):
        uops = lower(op.spec, ver=ver)
        sha[ver] = DveOpSpec(
            name=name, opcode=opcode, uops=uops, rd1_en=has_src1(op.spec)
        ).sha(ver)
    return op


def _build():
    """Build the per-core Bass program (identical on all 8 cores)."""
    mac_scan = _register_mac_scan()

    nc = bacc.Bacc("TRN2", target_bir_lowering=False, debug=False)
    f32 = mybir.dt.float32
    bf16 = mybir.dt.bfloat16

    xs = nc.dram_tensor("x", [BS, F], f32, kind="ExternalInput")
    whi = nc.dram_tensor("wrow_hi", [1, FP], bf16, kind="ExternalInput")
    wlo = nc.dram_tensor("wrow_lo", [1, FP], bf16, kind="ExternalInput")
    br = nc.dram_tensor("brep", [P, NGROUPS], f32, kind="ExternalInput")
    ys = nc.dram_tensor("y", [BS, NGROUPS], f32, kind="ExternalOutput")

    # tile t, partition p holds batch row t*P + p
    x_t = xs.ap().rearrange("(t p) f -> t p f", p=P)
    y_t = ys.ap().rearrange("(t p) g -> t p g", p=P)

    with tile.TileContext(nc) as tc:
        with (
            tc.tile_pool(name="const", bufs=1) as cpool,
            tc.tile_pool(name="q", bufs=NQ) as qpool,
            tc.tile_pool(name="o", bufs=6) as opool,
            tc.psum_pool(name="ps", bufs=4) as ppool,
        ):
            w_tile = cpool.tile([P, FP], f32)      # [16 zeros | W] broadcast
            b_tile = cpool.tile([P, NGROUPS], f32)
            whi_sb = cpool.tile([1, FP], bf16)
            wlo_sb = cpool.tile([1, FP], bf16)
            ones = cpool.tile([1, P], bf16)
            xbufs = [cpool.tile([P, FP], f32, name=f"xb{i}") for i in range(NXBUF)]

            # W rows + bias ride the input (sync) ring AHEAD of x: they
            # land with the first packets (~25+188 KB).
            nc.sync.dma_start(out=whi_sb[:], in_=whi.ap())
            nc.sync.dma_start(out=wlo_sb[:], in_=wlo.ap())
            nc.sync.dma_start(out=b_tile[:], in_=br.ap())
            nc.gpsimd.memset(ones[:], 1.0)
            # x-buffer prefixes: any finite garbage is fine under the zero
            # weights, but SBUF may hold NaN patterns at start -> memset once.
            for xb in xbufs:
                nc.gpsimd.memset(xb[:, 0:NMODELS], 0.0)

            # --- on-chip broadcast: PSUM[p, c] = 1.0*w_hi[c] + 1.0*w_lo[c]
            for c0 in range(0, FP, MMCH):
                cw = min(MMCH, FP - c0)
                ps = ppool.tile([P, MMCH], f32)
                nc.tensor.matmul(
                    ps[:, 0:cw], ones[:], whi_sb[:, c0 : c0 + cw],
                    start=True, stop=False,
                )
                nc.tensor.matmul(
                    ps[:, 0:cw], ones[:], wlo_sb[:, c0 : c0 + cw],
                    start=False, stop=True,
                )
                nc.scalar.copy(w_tile[:, c0 : c0 + cw], ps[:, 0:cw])

            def quarter_tile(i, ot, prefetched=None):
                """Ramp tile: 4 quarter scans + diffs, all on the DVE."""
                for q in range(NQ):
                    if prefetched is not None:
                        xq = prefetched[q]
                    else:
                        xq = qpool.tile([P, FQ], f32, name="xq", tag="xq")
                        nc.sync.dma_start(
                            out=xq[:], in_=x_t[i][:, q * FQ : (q + 1) * FQ]
                        )
                    wsl = slice(NMODELS + q * FQ, NMODELS + (q + 1) * FQ)
                    nc.vector._custom_dve(
                        mac_scan, out=xq[:], in0=xq[:], in1=w_tile[:, wsl]
                    )
                    hi = (
                        xq[:].rearrange("p (s n) -> p s n", n=NMODELS)[
                            :, :, NMODELS - 1 : NMODELS
                        ].rearrange("p s one -> p (s one)")
                    )
                    o = ot[:, q * GQ : (q + 1) * GQ]
                    nc.vector.tensor_copy(o[:, 0:1], hi[:, 0:1])
                    nc.vector.tensor_sub(o[:, 1:GQ], hi[:, 1:GQ], hi[:, 0 : GQ - 1])
                    nc.vector.tensor_add(
                        o, o, b_tile[:, q * GQ : (q + 1) * GQ]
                    )
                    nc.scalar.dma_start(
                        out=y_t[i][:, q * GQ : (q + 1) * GQ], in_=o
                    )

            t15_pre = None
            for i in range(NTILES):
                ot = opool.tile([P, NGROUPS], f32)
                if i == 0:
                    quarter_tile(i, ot)
                elif i == NTILES - 1:
                    quarter_tile(i, ot, prefetched=t15_pre)
                else:
                    xt = xbufs[(i - 1) % NXBUF]
                    nc.sync.dma_start(out=xt[:, NMODELS:FP], in_=x_t[i])
                    if i == 12:
                        # prefetch the last tile's quarters now: qpool bufs
                        # are free of the x-buffer recycling throttle, so
                        # these ride the stream at full rate and the tail
                        # scans never wait on data
                        t15_pre = []
                        for q in range(NQ):
                            xq = qpool.tile([P, FQ], f32, name="xq", tag="xq")
                            nc.sync.dma_start(
                                out=xq[:],
                                in_=x_t[NTILES - 1][:, q * FQ : (q + 1) * FQ],
                            )
                            t15_pre.append(xq)
                    # full-length scan incl. zero-weight prefix: cumsum
                    # starts at 0, so one SUB over the 369 block-ends
                    # yields all 368 group sums
                    nc.vector._custom_dve(
                        mac_scan, out=xt[:], in0=xt[:], in1=w_tile[:]
                    )
                    hi = (
                        xt[:].rearrange("p (s n) -> p s n", n=NMODELS)[
                            :, :, NMODELS - 1 : NMODELS
                        ].rearrange("p s one -> p (s one)")
                    )
                    nc.gpsimd.tensor_sub(ot[:], hi[:, 1 : NGROUPS + 1], hi[:, 0:NGROUPS])
                    nc.gpsimd.tensor_add(ot[:], ot[:], b_tile[:])
                    nc.scalar.dma_start(out=y_t[i], in_=ot[:])

    nc.compile()
    return nc


def get_nc():
    if "nc" not in _CACHE:
        _CACHE["nc"] = _build()
    return _CACHE["nc"]


def kernel(x: np.ndarray, W: np.ndarray, b: np.ndarray, trace: bool = False):
    x = np.asarray(x, dtype=np.float32)
    W = np.asarray(W, dtype=np.float32)
    b = np.asarray(b, dtype=np.float32)
    assert x.shape == (BATCH, NGROUPS, NMODELS)

    nc = get_nc()

    bf16 = ml_dtypes.bfloat16
    wflat = np.concatenate([np.zeros(NMODELS, np.float32), W.reshape(-1)])
    w_hi = wflat.astype(bf16)
    w_lo = (wflat - w_hi.astype(np.float32)).astype(bf16)
    wrow_hi = np.ascontiguousarray(w_hi.reshape(1, FP))
    wrow_lo = np.ascontiguousarray(w_lo.reshape(1, FP))
    brep = np.ascontiguousarray(np.broadcast_to(b, (P, NGROUPS)))

    x2 = x.reshape(BATCH, F)
    in_maps = [
        {
            "x": x2[c * BS : (c + 1) * BS],
            "wrow_hi": wrow_hi,
            "wrow_lo": wrow_lo,
            "brep": brep,
        }
        for c in range(NCORES)
    ]

    res = run_bass_kernel_spmd(
        nc, in_maps, core_ids=list(range(NCORES)), trace=trace
    )
    out = np.concatenate([res.results[c]["y"] for c in range(NCORES)], axis=0)
    if trace:
        kernel.last_exec_time_ns = res.exec_time_ns
        kernel.last_results = res
    return out


kernel.last_exec_time_ns = None
kernel.last_results = None


# revision 13
# speedup vs baseline: 1.0571x; 1.0571x over previous
"""Trainium2 Bass kernel for nn_EnsembleModel (grouped ensemble dot-product).

Computes out[b, g] = sum_n x[b, g, n] * W[g, n] + b[g] for
x: [16384, 368, 16] f32, W: [368, 16] f32, b: [368] f32.

Strategy: data-parallel over 8 NeuronCores (batch 16384 -> 8 x 2048).
Per core: batch rows on SBUF partitions (contiguous DMA). A custom DVE op
(MAC_SCAN: out = cumsum(x * w) along the free dim, one pass at 1 elem/cyc)
replaces the 2-pass mul+reduce; per-group sums are recovered as strided
differences of the cumulative sum, then bias.

The kernel is jointly DVE-bound (~100us of scans at 0.96 GHz) and
HBM-bound (~115us of x streaming at ~420 GB/s), so everything else is
kept off both paths:
 - W reaches the chip as two bf16 rows (hi+lo split, 23.6 KB) and is
   broadcast to all 128 partitions ON-CHIP: TensorE ones-matmuls
   accumulate hi+lo into PSUM (exact to ~2^-18), ACT copies to SBUF.
   (Replicating f32 W via HBM costs 3 MB = ~9us of stream time.)
 - Group-sum extraction runs on the idle GPSIMD engine, not the DVE:
   x tiles carry a 16-elem prefix whose *weights* are zero, so the
   cumulative sum starts at 0 and one 368-elem SUB yields all groups
   (prefix data itself may be garbage; only NaN/Inf must be avoided,
   hence a one-time memset per physical buffer).
 - Bias adds also run on GPSIMD; bias is replicated via HBM (188 KB).
 - Tiles 0/15 are processed in quarters fully on the DVE: tile 0 so
   compute starts as soon as the first W chunks land, tile 15 so the
   post-last-byte tail is one quarter-scan, not a full tile.
"""

import sys

for _p in ("/opt/trn_rl_repo", "/root/.axon_site/_ro/trn_rl_repo"):
    if _p not in sys.path:
        sys.path.append(_p)

import numpy as np
import ml_dtypes

import concourse.bacc as bacc
import concourse.bass as bass
import concourse.mybir as mybir
import concourse.tile as tile
from concourse.bass_utils import run_bass_kernel_spmd

BATCH = 16384
NGROUPS = 368
NMODELS = 16
NCORES = 8
BS = BATCH // NCORES          # 2048 batch rows per core
P = 128                       # SBUF partitions
F = NGROUPS * NMODELS         # 5888 x-elems per row
FP = F + NMODELS              # 5904 = 16-elem zero-weight prefix + x row
NTILES = BS // P              # 16 tiles of 128 rows

NQ = 4                        # quarters for the ramp tiles (0 and 15)
FQ = F // NQ                  # 1472
GQ = NGROUPS // NQ            # 92
MMCH = 512                    # matmul broadcast chunk (one PSUM bank of f32)
NXBUF = 5                     # persistent x buffers for the 14 full tiles

_CACHE = {}


def _register_mac_scan():
    """Register the fused multiply+cumsum custom DVE op at runtime."""
    import concourse.dve_ops as dve_ops
    from concourse.dve_ops import DveOp, OPS
    from concourse.dve_spec import AluOp, Spec, Src0, Src1, lower, scan
    from concourse.dve_spec import _has_src1 as has_src1
    from concourse.dve_uop import DveOpSpec

    name = "MAC_SCAN_ANT"
    for op in OPS:
        if op.name == name:
            return op

    def _ref(in0, in1, s0, s1, imm2):
        p = in0.shape[0]
        prod = (np.asarray(in0, np.float32) * np.asarray(in1, np.float32)).reshape(
            p, -1
        )
        return np.cumsum(prod, axis=1, dtype=np.float32).reshape(in0.shape)

    sha = {}
    op = DveOp(
        name,
        Spec(body=scan(AluOp.ADD, Src0 * Src1), reference=_ref),
        subdim=False,
        uops_sha=sha,
    )
    OPS.append(op)
    opcode = dve_ops._CUSTOM_DVE_ROW_BASE + len(OPS) - 1
    dve_ops._SUB_OPCODE_FOR_NAME[name] = opcode
    assert opcode < 0x20
    for ver in ("v3", "v4"):
        uops = lower(op.spec, ver=ver)
        sha[ver] = DveOpSpec(
            name=name, opcode=opcode, uops=uops, rd1_en=has_src1(op.spec)
        ).sha(ver)
    return op


def _build():
    """Build the per-core Bass program (identical on all 8 cores)."""
    mac_scan = _register_mac_scan()

    nc = bacc.Bacc("TRN2", target_bir_lowering=False, debug=False)
    f32 = mybir.dt.float32
    bf16 = mybir.dt.bfloat16

    xs = nc.dram_tensor("x", [BS, F], f32, kind="ExternalInput")
    whi = nc.dram_tensor("wrow_hi", [1, FP], bf16, kind="ExternalInput")
    wlo = nc.dram_tensor("wrow_lo", [1, FP], bf16, kind="ExternalInput")
    br = nc.dram_tensor("brep", [P, NGROUPS], f32, kind="ExternalInput")
    ys = nc.dram_tensor("y", [BS, NGROUPS], f32, kind="ExternalOutput")

    # tile t, partition p holds batch row t*P + p
    x_t = xs.ap().rearrange("(t p) f -> t p f", p=P)
    y_t = ys.ap().rearrange("(t p) g -> t p g", p=P)

    with tile.TileContext(nc) as tc:
        with (
            tc.tile_pool(name="const", bufs=1) as cpool,
            tc.tile_pool(name="q", bufs=NQ) as qpool,
            tc.tile_pool(name="o", bufs=6) as opool,
            tc.psum_pool(name="ps", bufs=4) as ppool,
        ):
            w_tile = cpool.tile([P, FP], f32)      # [16 zeros | W] broadcast
            b_tile = cpool.tile([P, NGROUPS], f32)
            whi_sb = cpool.tile([1, FP], bf16)
            wlo_sb = cpool.tile([1, FP], bf16)
            ones = cpool.tile([1, P], bf16)
            xbufs = [cpool.tile([P, FP], f32, name=f"xb{i}") for i in range(NXBUF)]

            # W rows + bias ride the input (sync) ring AHEAD of x: they
            # land with the first packets (~25+188 KB).
            nc.sync.dma_start(out=whi_sb[:], in_=whi.ap())
            nc.sync.dma_start(out=wlo_sb[:], in_=wlo.ap())
            nc.sync.dma_start(out=b_tile[:], in_=br.ap())
            nc.gpsimd.memset(ones[:], 1.0)
            # x-buffer prefixes: any finite garbage is fine under the zero
            # weights, but SBUF may hold NaN patterns at start -> memset once.
            for xb in xbufs:
                nc.gpsimd.memset(xb[:, 0:NMODELS], 0.0)

            # --- on-chip broadcast: PSUM[p, c] = 1.0*w_hi[c] + 1.0*w_lo[c]
            for c0 in range(0, FP, MMCH):
                cw = min(MMCH, FP - c0)
                ps = ppool.tile([P, MMCH], f32)
                nc.tensor.matmul(
                    ps[:, 0:cw], ones[:], whi_sb[:, c0 : c0 + cw],
                    start=True, stop=False,
                )
                nc.tensor.matmul(
                    ps[:, 0:cw], ones[:], wlo_sb[:, c0 : c0 + cw],
                    start=False, stop=True,
                )
                nc.scalar.copy(w_tile[:, c0 : c0 + cw], ps[:, 0:cw])

            def quarter_tile(i, ot):
                """Ramp tile: 4 quarter scans + diffs, all on the DVE."""
                for q in range(NQ):
                    xq = qpool.tile([P, FQ], f32, name="xq", tag="xq")
                    nc.sync.dma_start(out=xq[:], in_=x_t[i][:, q * FQ : (q + 1) * FQ])
                    wsl = slice(NMODELS + q * FQ, NMODELS + (q + 1) * FQ)
                    nc.vector._custom_dve(
                        mac_scan, out=xq[:], in0=xq[:], in1=w_tile[:, wsl]
                    )
                    hi = (
                        xq[:].rearrange("p (s n) -> p s n", n=NMODELS)[
                            :, :, NMODELS - 1 : NMODELS
                        ].rearrange("p s one -> p (s one)")
                    )
                    o = ot[:, q * GQ : (q + 1) * GQ]
                    nc.vector.tensor_copy(o[:, 0:1], hi[:, 0:1])
                    nc.vector.tensor_sub(o[:, 1:GQ], hi[:, 1:GQ], hi[:, 0 : GQ - 1])
                    nc.vector.tensor_add(
                        o, o, b_tile[:, q * GQ : (q + 1) * GQ]
                    )
                    nc.scalar.dma_start(
                        out=y_t[i][:, q * GQ : (q + 1) * GQ], in_=o
                    )

            for i in range(NTILES):
                ot = opool.tile([P, NGROUPS], f32)
                if i == 0 or i == NTILES - 1:
                    quarter_tile(i, ot)
                elif i == 1:
                    # single DMA, but the SCAN split into 4 chunk-gated
                    # segments: each needs only a prefix of w_tile, so the
                    # DVE rides the broadcast instead of idling ~4.5us for
                    # the full w_tile (no extra descriptors, unlike the
                    # failed quarter-DMA variant)
                    xt = xbufs[0]
                    nc.sync.dma_start(out=xt[:, NMODELS:FP], in_=x_t[i])
                    for q in range(NQ):
                        wsl = slice(NMODELS + q * FQ, NMODELS + (q + 1) * FQ)
                        seg = xt[:, wsl]
                        nc.vector._custom_dve(
                            mac_scan, out=seg, in0=seg, in1=w_tile[:, wsl]
                        )
                        hi = (
                            seg.rearrange("p (s n) -> p s n", n=NMODELS)[
                                :, :, NMODELS - 1 : NMODELS
                            ].rearrange("p s one -> p (s one)")
                        )
                        o = ot[:, q * GQ : (q + 1) * GQ]
                        nc.vector.tensor_copy(o[:, 0:1], hi[:, 0:1])
                        nc.vector.tensor_sub(
                            o[:, 1:GQ], hi[:, 1:GQ], hi[:, 0 : GQ - 1]
                        )
                        nc.vector.tensor_add(
                            o, o, b_tile[:, q * GQ : (q + 1) * GQ]
                        )
                    nc.scalar.dma_start(out=y_t[i], in_=ot[:])
                else:
                    xt = xbufs[(i - 1) % NXBUF]
                    nc.sync.dma_start(out=xt[:, NMODELS:FP], in_=x_t[i])
                    # full-length scan incl. zero-weight prefix: cumsum
                    # starts at 0, so one SUB over the 369 block-ends
                    # yields all 368 group sums
                    nc.vector._custom_dve(
                        mac_scan, out=xt[:], in0=xt[:], in1=w_tile[:]
                    )
                    hi = (
                        xt[:].rearrange("p (s n) -> p s n", n=NMODELS)[
                            :, :, NMODELS - 1 : NMODELS
                        ].rearrange("p s one -> p (s one)")
                    )
                    nc.gpsimd.tensor_sub(ot[:], hi[:, 1 : NGROUPS + 1], hi[:, 0:NGROUPS])
                    nc.gpsimd.tensor_add(ot[:], ot[:], b_tile[:])
                    nc.scalar.dma_start(out=y_t[i], in_=ot[:])

    nc.compile()
    return nc


def get_nc():
    if "nc" not in _CACHE:
        _CACHE["nc"] = _build()
    return _CACHE["nc"]


def kernel(x: np.ndarray, W: np.ndarray, b: np.ndarray, trace: bool = False):
    x = np.asarray(x, dtype=np.float32)
    W = np.asarray(W, dtype=np.float32)
    b = np.asarray(b, dtype=np.float32)
    assert x.shape == (BATCH, NGROUPS, NMODELS)

    nc = get_nc()

    bf16 = ml_dtypes.bfloat16
    wflat = np.concatenate([np.zeros(NMODELS, np.float32), W.reshape(-1)])
    w_hi = wflat.astype(bf16)
    w_lo = (wflat - w_hi.astype(np.float32)).astype(bf16)
    wrow_hi = np.ascontiguousarray(w_hi.reshape(1, FP))
    wrow_lo = np.ascontiguousarray(w_lo.reshape(1, FP))
    brep = np.ascontiguousarray(np.broadcast_to(b, (P, NGROUPS)))

    x2 = x.reshape(BATCH, F)
    in_maps = [
        {
            "x": x2[c * BS : (c + 1) * BS],
            "wrow_hi": wrow_hi,
            "wrow_lo": wrow_lo,
            "brep": brep,
        }
        for c in range(NCORES)
    ]

    res = run_bass_kernel_spmd(
        nc, in_maps, core_ids=list(range(NCORES)), trace=trace
    )
    out = np.concatenate([res.results[c]["y"] for c in range(NCORES)], axis=0)
    if trace:
        kernel.last_exec_time_ns = res.exec_time_ns
        kernel.last_results = res
    return out


kernel.last_exec_time_ns = None
kernel.last_results = None
